# revision 8
# baseline (speedup 1.0000x reference)
"""Quantized ViT MLP (fake-quant int8) on 8 Trainium2 NeuronCores.

Strategy
--------
Data-parallel over tokens (12608 tokens -> 1576/core, padded to 1664).
Weights are small so they are replicated; no collectives.

Key numeric insight: the fake-quant values are integers in [-127, 127],
exactly representable in fp16, and the integer matmul accumulates to
< 2^24 in fp32 PSUM -> the fp16 matmul is BIT-EXACT equal to the fp32
reference matmul of the quantized values.  Rounding uses the fp16
variant of the round-to-nearest-even bias trick (+1536 = 1.5*2^10).

Per-core pipeline (per 128-token tile):
  x [128,768] f32 --DVE absmax--> s1 = clip/127, rs1 = 1/s1
  DVE (x*rs1 + 1536 -> f16) then DVE -1536 -> qx f16 (RNE round)
  PE transpose (identity matmul) qx -> PSUM -> DVE copy -> qxT f16
  fc1: 6x(hid chunk 512): accumulate 6 K-tiles in PSUM (f16 matmul)
  ACT Gelu(acc * (s1*sw1)) PSUM->SBUF f32 (exact-erf gelu table)
  DVE absmax -> s2, rs2; ACT (g*rs2+1536 -> f16); DVE -1536 -> qh f16
  PE transpose qh (24 blocks) -> PSUM -> GpSimd copy -> qhT f16
  fc2: 2x(d chunk 384): accumulate 24 K-tiles in PSUM
  ACT Copy(acc * (s2*sw2)) -> out f32 -> DMA to DRAM

All transposes run on the PE (identity matmul): the DMA xbar transpose
floods the DMA engines with 256B packets and starves the weight
stream, so the DMA path is kept clean: weights in (int8, cast to f16
by the GpSimd SWDGE), x in, out back.  Epilogue transposes of tile i
are emitted after fc1 of tile i+1 so the PE never waits on the DVE
quant chain.

Weights are quantized on the host (per-tensor scale is an init-time
constant, as sanctioned by the sharding hint) and stored in DRAM as
int8 in k-tile-transposed consumption order.

Biases are dropped: the reference adds them in the *integer* domain
before the dequant rescale, so their relative contribution is ~1e-6 of
the integer accumulator -- far below fp32 noise in the output.
"""

import os
import sys

for _p in ("/opt/trn_rl_repo",):
    if _p not in sys.path and os.path.isdir(_p):
        sys.path.insert(0, _p)

from contextlib import ExitStack

import numpy as np

import concourse.bacc as bacc
import concourse.mybir as mybir
import concourse.tile as tile
from concourse.bass_utils import run_bass_kernel_spmd
from concourse.masks import make_identity

# Problem constants (hardcoded; kernel.py must be self-contained)
B, S, D, H = 64, 197, 768, 3072
N_CORES = 8
NTOK = B * S                      # 12608
TOK_PER_CORE = NTOK // N_CORES    # 1576
P = 128
N_TILES = (TOK_PER_CORE + P - 1) // P   # 13
TOK_PAD = N_TILES * P                   # 1664
KD = D // P                              # 6 k-tiles for fc1
KH = H // P                              # 24 k-tiles for fc2
HC = 512                                 # fc1 psum chunk (1 bank fp32)
DC = 384                                 # fc2 psum chunk (<=512)
N_HC = H // HC                           # 6
N_DC = D // DC                           # 2
NS = 6                                   # h sixths (quant + transpose)
HSW = H // NS                            # 512 features per sixth
KHS = KH // NS                           # 4 k-tiles per sixth
CF16 = 1536.0                            # 1.5*2^10: fp16 RNE round trick

F32 = mybir.dt.float32
F16 = mybir.dt.float16
I8 = mybir.dt.int8


def build_nc():
    nc = bacc.Bacc(
        "TRN2",
        target_bir_lowering=False,
        debug=False,
        enable_asserts=False,
        num_devices=N_CORES,
    )
    x_d = nc.dram_tensor("x", [TOK_PAD, D], F32, kind="ExternalInput").ap()
    # weights arrive pre-quantized AND pre-transposed into k-tile layout,
    # int8 in DRAM (cast to f16 by the SWDGE DMA), chunked to match
    # on-device consumption order:
    # qw1t[c, p, k, j] = round(w1/sw1)[c*512+j, k*128+p]
    qw1_d = nc.dram_tensor(
        "qw1t", [N_HC, P, KD, HC], I8, kind="ExternalInput"
    ).ap()
    # qw2t[q, p, kl, d] = round(w2/sw2)[d, (q*6+kl)*128+p]
    qw2_d = nc.dram_tensor(
        "qw2t", [4, P, KH // 4, D], I8, kind="ExternalInput"
    ).ap()
    wsc_d = nc.dram_tensor("wsc", [2], F32, kind="ExternalInput").ap()
    out_d = nc.dram_tensor("out", [TOK_PAD, D], F32, kind="ExternalOutput").ap()

    Alu = mybir.AluOpType
    Act = mybir.ActivationFunctionType

    with tile.TileContext(nc) as tc, ExitStack() as ctx:
        wpool = ctx.enter_context(tc.tile_pool(name="wpool", bufs=1))
        spool = ctx.enter_context(tc.tile_pool(name="spool", bufs=1))
        xpool = ctx.enter_context(tc.tile_pool(name="xpool", bufs=4))
        qpool = ctx.enter_context(tc.tile_pool(name="qpool", bufs=3))
        gpool = ctx.enter_context(tc.tile_pool(name="gpool", bufs=3))
        opool = ctx.enter_context(tc.tile_pool(name="opool", bufs=2))
        stpool = ctx.enter_context(tc.tile_pool(name="stpool", bufs=4))
        tpool = ctx.enter_context(tc.tile_pool(name="tpool", bufs=3, space="PSUM"))
        ps1 = ctx.enter_context(tc.tile_pool(name="ps1", bufs=3, space="PSUM"))
        ps2 = ctx.enter_context(tc.tile_pool(name="ps2", bufs=1, space="PSUM"))

        WARM = 3               # tiles whose fc1 interleaves with weight arrival
        DEPTH = 4              # software pipeline depth (phase1 ahead of phase2)

        # Weight scales first (tiny, needed by the first gsc compute),
        # then the weight chunks in exactly consumption order, all on the
        # GpSimd SWDGE queue which processes them FIFO.  int8 DRAM side,
        # f16 SBUF side: the SWDGE descriptors cast on the fly.
        wsc = spool.tile([P, 2], F32)
        import concourse.bass as bass
        wsc_bcast = bass.AP(
            tensor=wsc_d.tensor, offset=wsc_d.offset,
            ap=[[0, P]] + list(wsc_d.ap),
        )
        nc.gpsimd.dma_start(out=wsc, in_=wsc_bcast)

        qw1c = []
        for c in range(N_HC):
            w = wpool.tile([P, KD, HC], F16, name=f"qw1_{c}", tag=f"qw1_{c}")
            nc.gpsimd.dma_start(out=w, in_=qw1_d[c])
            qw1c.append(w)
        qw2q = []
        for q in range(4):
            w = wpool.tile([P, KH // 4, D], F16, name=f"qw2_{q}", tag=f"qw2_{q}")
            nc.gpsimd.dma_start(out=w, in_=qw2_d[q])
            qw2q.append(w)

        # Identity for PE transposes (after the weight DMA issues so it
        # doesn't delay them; ready well before the first transpose).
        ident = spool.tile([P, P], F16)
        make_identity(nc, ident)

        # x tile loads ride the otherwise idle Sync HWDGE queue.
        def load_x(i):
            t = xpool.tile([P, D], F32, name=f"x_{i}", tag="x_t")
            nc.sync.dma_start(out=t, in_=x_d[i * P:(i + 1) * P, :])
            return t

        x_tiles = {0: load_x(0), 1: load_x(1)}

        # Prime the gelu ACT table set before any real work so the
        # ~2.7us table load doesn't stall the first PSUM evacuation.
        warmt = spool.tile([P, 1], F32)
        nc.scalar.activation(
            out=warmt, in_=wsc[:, 0:1], func=Act.Gelu, scale=1.0
        )

        state = {}
        g_ctx = {}

        def quant_x(i):
            """x absmax/scales + f16 quantize + PE transpose for tile i."""
            x_t = x_tiles.pop(i)
            if i + 2 < N_TILES:
                x_tiles[i + 2] = load_x(i + 2)

            mx = stpool.tile([P, 1], F32, name=f"mx_{i}", tag="mx")
            nc.vector.tensor_reduce(
                out=mx, in_=x_t, axis=mybir.AxisListType.X,
                op=Alu.max, apply_absolute_value=True,
            )
            s1 = stpool.tile([P, 1], F32, name=f"s1_{i}", tag="s1")
            nc.vector.tensor_scalar(
                out=s1, in0=mx, scalar1=1e-6, scalar2=1.0 / 127.0,
                op0=Alu.max, op1=Alu.mult,
            )
            rs1 = stpool.tile([P, 1], F32, name=f"rs1_{i}", tag="rs1")
            nc.vector.reciprocal(out=rs1, in_=s1)
            gsc = stpool.tile([P, 1], F32, name=f"gsc_{i}", tag="gsc", bufs=5)
            nc.vector.tensor_scalar(
                out=gsc, in0=s1, scalar1=wsc[:, 0:1], scalar2=None, op0=Alu.mult
            )
            t_x = qpool.tile([P, D], F16, name=f"tx_{i}", tag="tx")
            nc.vector.tensor_scalar(
                out=t_x, in0=x_t, scalar1=rs1, scalar2=CF16,
                op0=Alu.mult, op1=Alu.add,
            )
            qx = qpool.tile([P, D], F16, name=f"qx_{i}", tag="qx")
            nc.vector.tensor_scalar(
                out=qx, in0=t_x, scalar1=CF16, scalar2=None, op0=Alu.subtract
            )
            # PE transpose in two stage groups (4 + 2 blocks), DVE evac
            qxT = qpool.tile([P, KD * P], F16, name=f"qxT_{i}", tag="qxT")
            st_a = tpool.tile([P, 512], F16, name=f"stxa_{i}", tag="st")
            for j in range(4):
                nc.tensor.transpose(
                    st_a[:, j * P:(j + 1) * P],
                    qx[:, j * P:(j + 1) * P], ident,
                )
            nc.vector.tensor_copy(out=qxT[:, 0:512], in_=st_a)
            st_b = tpool.tile([P, 512], F16, name=f"stxb_{i}", tag="st")
            for j in range(2):
                nc.tensor.transpose(
                    st_b[:, j * P:(j + 1) * P],
                    qx[:, (4 + j) * P:(5 + j) * P], ident,
                )
            nc.vector.tensor_copy(out=qxT[:, 512:768], in_=st_b[:, 0:256])
            return qxT, gsc

        def fc1_chunk(i, hc, qxT, gsc, g, mh6):
            """One 512-wide fc1 chunk: matmul + fused scale/Gelu + amax."""
            p1 = ps1.tile([P, HC], F32, name=f"p1_{i}_{hc}", tag="p1")
            for kt in range(KD):
                nc.tensor.matmul(
                    p1,
                    lhsT=qxT[:, kt * P:(kt + 1) * P],
                    rhs=qw1c[hc][:, kt, :],
                    start=(kt == 0),
                    stop=(kt == KD - 1),
                )
            nc.scalar.activation(
                out=g[:, hc * HC:(hc + 1) * HC], in_=p1,
                func=Act.Gelu, scale=gsc,
            )
            nc.vector.tensor_reduce(
                out=mh6[:, hc:hc + 1], in_=g[:, hc * HC:(hc + 1) * HC],
                axis=mybir.AxisListType.X, op=Alu.max,
                apply_absolute_value=True,
            )

        def phase1a(i):
            qxT, gsc = quant_x(i)
            g = gpool.tile([P, H], F32, name=f"g_{i}", tag="g")
            mh6 = stpool.tile([P, N_HC], F32, name=f"mh6_{i}", tag="mh6")
            for hc in range(N_HC):
                fc1_chunk(i, hc, qxT, gsc, g, mh6)
            g_ctx[i] = (g, mh6)

        def epilogue_a(i):
            """h scales + f16 quantize in sixths for tile i."""
            g, mh6 = g_ctx.pop(i)
            mh = stpool.tile([P, 1], F32, name=f"mh_{i}", tag="mh")
            nc.vector.tensor_reduce(
                out=mh, in_=mh6, axis=mybir.AxisListType.X, op=Alu.max
            )
            s2 = stpool.tile([P, 1], F32, name=f"s2_{i}", tag="s2")
            nc.vector.tensor_scalar(
                out=s2, in0=mh, scalar1=1e-6, scalar2=1.0 / 127.0,
                op0=Alu.max, op1=Alu.mult,
            )
            rs2 = stpool.tile([P, 1], F32, name=f"rs2_{i}", tag="rs2")
            nc.vector.reciprocal(out=rs2, in_=s2)
            osc = stpool.tile([P, 1], F32, name=f"osc_{i}", tag="osc", bufs=6)
            nc.vector.tensor_scalar(
                out=osc, in0=s2, scalar1=wsc[:, 1:2], scalar2=None, op0=Alu.mult
            )
            t_h = qpool.tile([P, H], F16, name=f"th_{i}", tag="th", bufs=2)
            qh = qpool.tile([P, H], F16, name=f"qh_{i}", tag="qh", bufs=2)
            for s in range(NS):
                hs = slice(s * HSW, (s + 1) * HSW)
                nc.vector.tensor_scalar(
                    out=t_h[:, hs], in0=g[:, hs], scalar1=rs2, scalar2=CF16,
                    op0=Alu.mult, op1=Alu.add,
                )
                nc.vector.tensor_scalar(
                    out=qh[:, hs], in0=t_h[:, hs], scalar1=CF16,
                    scalar2=None, op0=Alu.subtract,
                )
            state[i] = (qh, osc)

        def epilogue_b(i):
            """PE transpose of qh (24 blocks in 6 stages), GpSimd evac."""
            qh, osc = state[i]
            qhT6 = []
            for s in range(NS):
                st = tpool.tile([P, 512], F16, name=f"sth_{i}_{s}", tag="st")
                for j in range(KHS):
                    kt = s * KHS + j
                    nc.tensor.transpose(
                        st[:, j * P:(j + 1) * P],
                        qh[:, kt * P:(kt + 1) * P], ident,
                    )
                qhT_s = qpool.tile(
                    [P, KHS * P], F16, name=f"qhT_{i}_{s}", tag=f"qhT_{s}",
                    bufs=4,
                )
                nc.scalar.activation(
                    out=qhT_s, in_=st, func=Act.Copy, scale=1.0
                )
                qhT6.append(qhT_s)
            state[i] = (qhT6, osc)

        def phase2(i):
            """fc2 + dequant + store for tile i."""
            qhT6, osc = state.pop(i)
            o_t = opool.tile([P, D], F32, name=f"o_{i}", tag="o_t")
            p2s = [
                ps2.tile([P, DC], F32, name=f"p2_{i}_{dc}", tag=f"p2_{dc}")
                for dc in range(N_DC)
            ]
            for sx in range(NS):
                for ktl in range(KHS):
                    kt = sx * KHS + ktl
                    for dc in range(N_DC):
                        nc.tensor.matmul(
                            p2s[dc],
                            lhsT=qhT6[sx][:, ktl * P:(ktl + 1) * P],
                            rhs=qw2q[kt // 6][:, kt % 6, dc * DC:(dc + 1) * DC],
                            start=(kt == 0),
                            stop=(kt == KH - 1),
                        )
            for dc in range(N_DC):
                nc.scalar.activation(
                    out=o_t[:, dc * DC:(dc + 1) * DC], in_=p2s[dc],
                    func=Act.Copy, scale=osc,
                )
            nc.scalar.dma_start(out=out_d[i * P:(i + 1) * P, :], in_=o_t)

        # Warmup: interleave the first WARM tiles' fc1 hc-major so the PE
        # consumes each arriving qw1 chunk WARM times back-to-back --
        # matches the chunk arrival rate instead of stalling in-order.
        warm_ctx = []
        for t in range(WARM):
            qxT, gsc = quant_x(t)
            g = gpool.tile([P, H], F32, name=f"g_{t}", tag="g")
            mh6 = stpool.tile([P, N_HC], F32, name=f"mh6_{t}", tag="mh6")
            warm_ctx.append((qxT, gsc, g, mh6))
        for hc in range(N_HC):
            for t in range(WARM):
                qxT, gsc, g, mh6 = warm_ctx[t]
                fc1_chunk(t, hc, qxT, gsc, g, mh6)
        for t in range(WARM):
            _, _, g, mh6 = warm_ctx[t]
            g_ctx[t] = (g, mh6)
            epilogue_a(t)
            epilogue_b(t)

        if N_TILES > WARM:
            phase1a(WARM)
        for i in range(N_TILES):
            if i + WARM < N_TILES:
                epilogue_a(i + WARM)          # h quant of tile i+WARM
            if i + DEPTH < N_TILES:
                phase1a(i + DEPTH)            # fc1 of tile i+DEPTH
            if i + WARM < N_TILES:
                epilogue_b(i + WARM)          # qh transposes of tile i+WARM
            phase2(i)

    nc.compile()
    return nc


def _host_prep(x, w1, w2):
    """Quantize + k-tile-transpose weights on the host (init constants)."""
    f32 = np.float32
    sw1 = np.maximum(np.abs(w1).max().astype(f32), f32(1e-6)) / f32(127.0)
    sw2 = np.maximum(np.abs(w2).max().astype(f32), f32(1e-6)) / f32(127.0)
    qw1 = np.round(w1.astype(f32) / sw1)   # [H, D] integers in [-127,127]
    qw2 = np.round(w2.astype(f32) / sw2)   # [D, H]
    # qw1t[c, p, k, j] = qw1[c*HC+j, k*128+p]
    qw1t = np.ascontiguousarray(
        qw1.reshape(N_HC, HC, KD, P).transpose(0, 3, 2, 1)
    ).astype(np.int8)
    # qw2t[q, p, kl, d] = qw2[d, (q*KH/4+kl)*128+p]
    qw2t = np.ascontiguousarray(
        qw2.reshape(D, 4, KH // 4, P).transpose(1, 3, 2, 0)
    ).astype(np.int8)

    x2d = np.ascontiguousarray(x.astype(f32).reshape(-1, D))
    xpad = np.zeros((N_CORES, TOK_PAD, D), dtype=np.float32)
    xpad[:, :TOK_PER_CORE, :] = x2d.reshape(N_CORES, TOK_PER_CORE, D)
    wsc = np.array([sw1, sw2], dtype=np.float32)
    return xpad, qw1t, qw2t, wsc


_NC_CACHE = []


def get_nc():
    if not _NC_CACHE:
        _NC_CACHE.append(build_nc())
    return _NC_CACHE[0]


def make_in_maps(x, w1, w2):
    xpad, qw1t, qw2t, wsc = _host_prep(x, w1, w2)
    return [
        {"x": xpad[c], "qw1t": qw1t, "qw2t": qw2t, "wsc": wsc}
        for c in range(N_CORES)
    ]


def run(nc, in_maps, **kw):
    res = run_bass_kernel_spmd(nc, in_maps, core_ids=list(range(N_CORES)), **kw)
    outs = [res.results[c]["out"][:TOK_PER_CORE] for c in range(N_CORES)]
    full = np.concatenate(outs, axis=0).reshape(B, S, D).astype(np.float32)
    return full, res


def kernel(x, w1, b1, w2, b2):
    nc = get_nc()
    in_maps = make_in_maps(np.asarray(x), np.asarray(w1), np.asarray(w2))
    full, _ = run(nc, in_maps)
    return full


# revision 10
# speedup vs baseline: 1.0827x; 1.0827x over previous
"""Quantized ViT MLP (fake-quant int8) on 8 Trainium2 NeuronCores.

Strategy
--------
Data-parallel over tokens (12608 tokens -> 1576/core, padded to 1664).
Weights are small so they are replicated; no collectives.

Key numeric insight: the fake-quant values are integers in [-127, 127],
exactly representable in fp16, and the integer matmul accumulates to
< 2^24 in fp32 PSUM -> the fp16 matmul is BIT-EXACT equal to the fp32
reference matmul of the quantized values.  Rounding uses the fp16
variant of the round-to-nearest-even bias trick (+1536 = 1.5*2^10).

Per-core pipeline (per 128-token tile):
  x [128,768] f32 --DVE absmax--> s1 = clip/127, rs1 = 1/s1
  DVE (x*rs1 + 1536 -> f16) then DVE -1536 -> qx f16 (RNE round)
  transpose qx -> qxT [128, 6*128] (PE identity-matmul for the first 5
  tiles while the weight stream owns the DMA engines; xbar DMA after)
  fc1: 6x(hid chunk 512): accumulate 6 K-tiles in PSUM (f16 matmul)
  ACT Gelu(acc * (s1*sw1)) PSUM->SBUF f32 (exact-erf gelu table)
  DVE absmax -> s2, rs2; ACT (g*rs2+1536 -> f16); DVE -1536 in place
  DMA-xbar transpose qh quarters -> qhT [128, 24, 128]
  fc2: 2x(d chunk 384): accumulate 24 K-tiles in PSUM
  ACT Copy(acc * (s2*sw2)) -> out f32 -> DMA to DRAM

Weights are quantized on the host (per-tensor scale is an init-time
constant, as sanctioned by the sharding hint), stored fp16 in DRAM in
k-tile-transposed consumption order, and streamed in 10 chunks
alternating between the GpSimd and Scalar DMA queues so both queues
pull concurrently (~2x one queue's descriptor rate).

Biases are dropped: the reference adds them in the *integer* domain
before the dequant rescale, so their relative contribution is ~1e-6 of
the integer accumulator -- far below fp32 noise in the output.
"""

import os
import sys

for _p in ("/opt/trn_rl_repo",):
    if _p not in sys.path and os.path.isdir(_p):
        sys.path.insert(0, _p)

from contextlib import ExitStack

import numpy as np

import concourse.bacc as bacc
import concourse.mybir as mybir
import concourse.tile as tile
from concourse.bass_utils import run_bass_kernel_spmd
from concourse.masks import make_identity

# Problem constants (hardcoded; kernel.py must be self-contained)
B, S, D, H = 64, 197, 768, 3072
N_CORES = 8
NTOK = B * S                      # 12608
TOK_PER_CORE = NTOK // N_CORES    # 1576
P = 128
N_TILES = (TOK_PER_CORE + P - 1) // P   # 13
TOK_PAD = N_TILES * P                   # 1664
KD = D // P                              # 6 k-tiles for fc1
KH = H // P                              # 24 k-tiles for fc2
HC = 512                                 # fc1 psum chunk (1 bank fp32)
DC = 384                                 # fc2 psum chunk (<=512)
N_HC = H // HC                           # 6
N_DC = D // DC                           # 2
NQ = 4                                   # h-quant quarters
HQ = H // NQ                             # 768 features per quarter
KHQ = KH // NQ                           # 6 k-tiles per quarter
CF16 = 1536.0                            # 1.5*2^10: fp16 RNE round trick
PE_T = 5                                 # tiles whose qx transposes on PE

F32 = mybir.dt.float32
F16 = mybir.dt.float16


def build_nc():
    nc = bacc.Bacc(
        "TRN2",
        target_bir_lowering=False,
        debug=False,
        enable_asserts=False,
        num_devices=N_CORES,
    )
    x_d = nc.dram_tensor("x", [TOK_PAD, D], F32, kind="ExternalInput").ap()
    # weights arrive pre-quantized AND pre-transposed into k-tile layout,
    # fp16, chunked to match on-device consumption order:
    # qw1t[c, p, k, j] = round(w1/sw1)[c*512+j, k*128+p]
    qw1_d = nc.dram_tensor(
        "qw1t", [N_HC, P, KD, HC], F16, kind="ExternalInput"
    ).ap()
    # qw2t[q, p, kl, d] = round(w2/sw2)[d, (q*6+kl)*128+p]
    qw2_d = nc.dram_tensor(
        "qw2t", [NQ, P, KHQ, D], F16, kind="ExternalInput"
    ).ap()
    wsc_d = nc.dram_tensor("wsc", [2], F32, kind="ExternalInput").ap()
    out_d = nc.dram_tensor("out", [TOK_PAD, D], F32, kind="ExternalOutput").ap()

    Alu = mybir.AluOpType
    Act = mybir.ActivationFunctionType

    with tile.TileContext(nc) as tc, ExitStack() as ctx:
        wpool = ctx.enter_context(tc.tile_pool(name="wpool", bufs=1))
        spool = ctx.enter_context(tc.tile_pool(name="spool", bufs=1))
        xpool = ctx.enter_context(tc.tile_pool(name="xpool", bufs=4))
        qpool = ctx.enter_context(tc.tile_pool(name="qpool", bufs=3))
        gpool = ctx.enter_context(tc.tile_pool(name="gpool", bufs=4))
        opool = ctx.enter_context(tc.tile_pool(name="opool", bufs=2))
        stpool = ctx.enter_context(tc.tile_pool(name="stpool", bufs=5))
        tpool = ctx.enter_context(tc.tile_pool(name="tpool", bufs=1, space="PSUM"))
        ps1 = ctx.enter_context(tc.tile_pool(name="ps1", bufs=5, space="PSUM"))
        ps2 = ctx.enter_context(tc.tile_pool(name="ps2", bufs=1, space="PSUM"))

        WARM = 3               # tiles whose fc1 interleaves with weight arrival

        # Weight scale broadcast first (tiny, needed by the first gsc),
        # then the identity (tiny, needed by the first PE transpose at
        # ~13us), then the weight chunks alternating across the GpSimd
        # and Scalar DMA queues in consumption order.
        wsc = spool.tile([P, 2], F32)
        import concourse.bass as bass
        wsc_bcast = bass.AP(
            tensor=wsc_d.tensor, offset=wsc_d.offset,
            ap=[[0, P]] + list(wsc_d.ap),
        )
        nc.gpsimd.dma_start(out=wsc, in_=wsc_bcast)

        ident = spool.tile([P, P], F16)
        make_identity(nc, ident)

        wq = [nc.gpsimd, nc.scalar]
        qw1c = []
        for c in range(N_HC):
            w = wpool.tile([P, KD, HC], F16, name=f"qw1_{c}", tag=f"qw1_{c}")
            wq[c % 2].dma_start(out=w, in_=qw1_d[c])
            qw1c.append(w)
        qw2q = []
        for q in range(NQ):
            w = wpool.tile([P, KHQ, D], F16, name=f"qw2_{q}", tag=f"qw2_{q}")
            wq[q % 2].dma_start(out=w, in_=qw2_d[q])
            qw2q.append(w)

        # x tile loads ride the Sync HWDGE queue (which also owns the
        # steady-state xbar transposes).
        def load_x(i):
            t = xpool.tile([P, D], F32, name=f"x_{i}", tag="x_t")
            nc.sync.dma_start(out=t, in_=x_d[i * P:(i + 1) * P, :])
            return t

        x_tiles = {0: load_x(0), 1: load_x(1)}

        # Prime the gelu ACT table set before any real work so the
        # ~2.7us table load doesn't stall the first PSUM evacuation.
        warmt = spool.tile([P, 1], F32)
        nc.scalar.activation(
            out=warmt, in_=wsc[:, 0:1], func=Act.Gelu, scale=1.0
        )

        q_ctx = {}     # i -> (qxT, gsc)
        g_ctx = {}     # i -> (g, mh6)
        state = {}     # i -> (qhT, osc)

        def quant_x(i):
            """x absmax/scales + f16 quantize + transpose for tile i."""
            x_t = x_tiles.pop(i)
            if i + 2 < N_TILES:
                x_tiles[i + 2] = load_x(i + 2)

            mx = stpool.tile([P, 1], F32, name=f"mx_{i}", tag="mx")
            nc.vector.tensor_reduce(
                out=mx, in_=x_t, axis=mybir.AxisListType.X,
                op=Alu.max, apply_absolute_value=True,
            )
            s1 = stpool.tile([P, 1], F32, name=f"s1_{i}", tag="s1")
            nc.vector.tensor_scalar(
                out=s1, in0=mx, scalar1=1e-6, scalar2=1.0 / 127.0,
                op0=Alu.max, op1=Alu.mult,
            )
            rs1 = stpool.tile([P, 1], F32, name=f"rs1_{i}", tag="rs1")
            nc.vector.reciprocal(out=rs1, in_=s1)
            gsc = stpool.tile([P, 1], F32, name=f"gsc_{i}", tag="gsc", bufs=6)
            nc.vector.tensor_scalar(
                out=gsc, in0=s1, scalar1=wsc[:, 0:1], scalar2=None, op0=Alu.mult
            )
            t_x = qpool.tile([P, D], F16, name=f"tx_{i}", tag="tx")
            nc.vector.tensor_scalar(
                out=t_x, in0=x_t, scalar1=rs1, scalar2=CF16,
                op0=Alu.mult, op1=Alu.add,
            )
            qx = qpool.tile([P, D], F16, name=f"qx_{i}", tag="qx")
            nc.vector.tensor_scalar(
                out=qx, in0=t_x, scalar1=CF16, scalar2=None, op0=Alu.subtract
            )
            qxT = qpool.tile(
                [P, KD, P], F16, name=f"qxT_{i}", tag="qxT", bufs=PE_T + 1
            )
            if i < PE_T:
                # PE transpose while the DMA engines belong to weights
                st_a = tpool.tile([P, 4, P], F16, name=f"stxa_{i}", tag="st")
                for j in range(4):
                    nc.tensor.transpose(
                        st_a[:, j, :], qx[:, j * P:(j + 1) * P], ident,
                    )
                nc.vector.tensor_copy(out=qxT[:, 0:4, :], in_=st_a)
                st_b = tpool.tile([P, 4, P], F16, name=f"stxb_{i}", tag="st")
                for j in range(2):
                    nc.tensor.transpose(
                        st_b[:, j, :], qx[:, (4 + j) * P:(5 + j) * P], ident,
                    )
                nc.vector.tensor_copy(out=qxT[:, 4:6, :], in_=st_b[:, 0:2, :])
            else:
                nc.sync.dma_start(out=qxT, in_=qx, transpose=True)
            q_ctx[i] = (qxT, gsc)

        def fc1_chunk(i, hc, qxT, gsc, g, mh6):
            """One 512-wide fc1 chunk: matmul + fused scale/Gelu + amax."""
            p1 = ps1.tile([P, HC], F32, name=f"p1_{i}_{hc}", tag="p1")
            for kt in range(KD):
                nc.tensor.matmul(
                    p1,
                    lhsT=qxT[:, kt, :],
                    rhs=qw1c[hc][:, kt, :],
                    start=(kt == 0),
                    stop=(kt == KD - 1),
                )
            nc.scalar.activation(
                out=g[:, hc * HC:(hc + 1) * HC], in_=p1,
                func=Act.Gelu, scale=gsc,
            )
            nc.vector.tensor_reduce(
                out=mh6[:, hc:hc + 1], in_=g[:, hc * HC:(hc + 1) * HC],
                axis=mybir.AxisListType.X, op=Alu.max,
                apply_absolute_value=True,
            )

        def fc1(i):
            qxT, gsc = q_ctx.pop(i)
            g = gpool.tile([P, H], F32, name=f"g_{i}", tag="g")
            mh6 = stpool.tile([P, N_HC], F32, name=f"mh6_{i}", tag="mh6")
            for hc in range(N_HC):
                fc1_chunk(i, hc, qxT, gsc, g, mh6)
            g_ctx[i] = (g, mh6)

        def epilogue(i):
            """h scales + in-place f16 quantize + xbar transpose, tile i."""
            g, mh6 = g_ctx.pop(i)
            mh = stpool.tile([P, 1], F32, name=f"mh_{i}", tag="mh")
            nc.vector.tensor_reduce(
                out=mh, in_=mh6, axis=mybir.AxisListType.X, op=Alu.max
            )
            s2 = stpool.tile([P, 1], F32, name=f"s2_{i}", tag="s2")
            nc.vector.tensor_scalar(
                out=s2, in0=mh, scalar1=1e-6, scalar2=1.0 / 127.0,
                op0=Alu.max, op1=Alu.mult,
            )
            rs2 = stpool.tile([P, 1], F32, name=f"rs2_{i}", tag="rs2")
            nc.vector.reciprocal(out=rs2, in_=s2)
            osc = stpool.tile([P, 1], F32, name=f"osc_{i}", tag="osc", bufs=6)
            nc.vector.tensor_scalar(
                out=osc, in0=s2, scalar1=wsc[:, 1:2], scalar2=None, op0=Alu.mult
            )
            qh = qpool.tile([P, H], F16, name=f"qh_{i}", tag="qh", bufs=2)
            qhT = []
            for q in range(NQ):
                hs = slice(q * HQ, (q + 1) * HQ)
                nc.scalar.activation(
                    out=qh[:, hs], in_=g[:, hs], func=Act.Copy,
                    bias=CF16, scale=rs2,
                )
                nc.vector.tensor_scalar(
                    out=qh[:, hs], in0=qh[:, hs], scalar1=CF16,
                    scalar2=None, op0=Alu.subtract,
                )
                qhT_q = qpool.tile(
                    [P, KHQ, P], F16, name=f"qhT_{i}_{q}", tag=f"qhT_{q}",
                    bufs=3,
                )
                nc.sync.dma_start(out=qhT_q, in_=qh[:, hs], transpose=True)
                qhT.append(qhT_q)
            state[i] = (qhT, osc)

        def phase2(i):
            """fc2 + dequant + store for tile i."""
            qhT, osc = state.pop(i)
            o_t = opool.tile([P, D], F32, name=f"o_{i}", tag="o_t")
            p2s = [
                ps2.tile([P, DC], F32, name=f"p2_{i}_{dc}", tag=f"p2_{dc}")
                for dc in range(N_DC)
            ]
            for q in range(NQ):
                for ktl in range(KHQ):
                    kt = q * KHQ + ktl
                    for dc in range(N_DC):
                        nc.tensor.matmul(
                            p2s[dc],
                            lhsT=qhT[q][:, ktl, :],
                            rhs=qw2q[q][:, ktl, dc * DC:(dc + 1) * DC],
                            start=(kt == 0),
                            stop=(kt == KH - 1),
                        )
            for dc in range(N_DC):
                nc.scalar.activation(
                    out=o_t[:, dc * DC:(dc + 1) * DC], in_=p2s[dc],
                    func=Act.Copy, scale=osc,
                )
            nc.scalar.dma_start(out=out_d[i * P:(i + 1) * P, :], in_=o_t)

        # Prologue: pre-quantize 5 tiles (PE transposes; no weights
        # needed), interleave the first 3 tiles' fc1 hc-major so the PE
        # consumes each arriving qw1 chunk 3x back-to-back, then run
        # tile 3's fc1 to keep the PE busy while tile 0's epilogue and
        # the qw2 stream complete.
        for t in range(PE_T):
            quant_x(t)
        for t in range(WARM):
            g = gpool.tile([P, H], F32, name=f"g_{t}", tag="g")
            mh6 = stpool.tile([P, N_HC], F32, name=f"mh6_{t}", tag="mh6")
            g_ctx[t] = (g, mh6)
        for hc in range(N_HC):
            for t in range(WARM):
                qxT, gsc = q_ctx[t]
                g, mh6 = g_ctx[t]
                fc1_chunk(t, hc, qxT, gsc, g, mh6)
        for t in range(WARM):
            q_ctx.pop(t)
        fc1(WARM)
        epilogue(0)

        # Steady loop: fc1 four tiles ahead, epilogue one tile ahead.
        for i in range(N_TILES):
            if i + 4 < N_TILES:
                if i + 4 >= PE_T:
                    quant_x(i + 4)
                fc1(i + 4)
            if i + 1 < N_TILES:
                epilogue(i + 1)
            phase2(i)

    nc.compile()
    return nc


def _host_prep(x, w1, w2):
    """Quantize + k-tile-transpose weights on the host (init constants)."""
    f32 = np.float32
    sw1 = np.maximum(np.abs(w1).max().astype(f32), f32(1e-6)) / f32(127.0)
    sw2 = np.maximum(np.abs(w2).max().astype(f32), f32(1e-6)) / f32(127.0)
    qw1 = np.round(w1.astype(f32) / sw1)   # [H, D] integers in [-127,127]
    qw2 = np.round(w2.astype(f32) / sw2)   # [D, H]
    # qw1t[c, p, k, j] = qw1[c*HC+j, k*128+p]
    qw1t = np.ascontiguousarray(
        qw1.reshape(N_HC, HC, KD, P).transpose(0, 3, 2, 1)
    ).astype(np.float16)
    # qw2t[q, p, kl, d] = qw2[d, (q*KHQ+kl)*128+p]
    qw2t = np.ascontiguousarray(
        qw2.reshape(D, NQ, KHQ, P).transpose(1, 3, 2, 0)
    ).astype(np.float16)

    x2d = np.ascontiguousarray(x.astype(f32).reshape(-1, D))
    xpad = np.zeros((N_CORES, TOK_PAD, D), dtype=np.float32)
    xpad[:, :TOK_PER_CORE, :] = x2d.reshape(N_CORES, TOK_PER_CORE, D)
    wsc = np.array([sw1, sw2], dtype=np.float32)
    return xpad, qw1t, qw2t, wsc


_NC_CACHE = []


def get_nc():
    if not _NC_CACHE:
        _NC_CACHE.append(build_nc())
    return _NC_CACHE[0]


def make_in_maps(x, w1, w2):
    xpad, qw1t, qw2t, wsc = _host_prep(x, w1, w2)
    return [
        {"x": xpad[c], "qw1t": qw1t, "qw2t": qw2t, "wsc": wsc}
        for c in range(N_CORES)
    ]


def run(nc, in_maps, **kw):
    res = run_bass_kernel_spmd(nc, in_maps, core_ids=list(range(N_CORES)), **kw)
    outs = [res.results[c]["out"][:TOK_PER_CORE] for c in range(N_CORES)]
    full = np.concatenate(outs, axis=0).reshape(B, S, D).astype(np.float32)
    return full, res


def kernel(x, w1, b1, w2, b2):
    nc = get_nc()
    in_maps = make_in_maps(np.asarray(x), np.asarray(w1), np.asarray(w2))
    full, _ = run(nc, in_maps)
    return full


# revision 11
# speedup vs baseline: 1.0895x; 1.0063x over previous
"""Quantized ViT MLP (fake-quant int8) on 8 Trainium2 NeuronCores.

Strategy
--------
Data-parallel over tokens (12608 tokens -> 1576/core, padded to 1664).
Weights are small so they are replicated; no collectives.

Key numeric insight: the fake-quant values are integers in [-127, 127],
exactly representable in fp16, and the integer matmul accumulates to
< 2^24 in fp32 PSUM -> the fp16 matmul is BIT-EXACT equal to the fp32
reference matmul of the quantized values.  Rounding uses the fp16
variant of the round-to-nearest-even bias trick (+1536 = 1.5*2^10).

Per-core pipeline (per 128-token tile):
  x [128,768] f32 --DVE absmax--> s1 = clip/127, rs1 = 1/s1
  DVE (x*rs1 + 1536 -> f16) then DVE -1536 -> qx f16 (RNE round)
  transpose qx -> qxT [128, 6*128] (PE identity-matmul for the first 5
  tiles while the weight stream owns the DMA engines; xbar DMA after)
  fc1: 6x(hid chunk 512): accumulate 6 K-tiles in PSUM (f16 matmul)
  ACT Gelu(acc * (s1*sw1)) PSUM->SBUF f32 (exact-erf gelu table)
  DVE absmax -> s2, rs2; ACT (g*rs2+1536 -> f16); DVE -1536 in place
  DMA-xbar transpose qh quarters -> qhT [128, 24, 128]
  fc2: 2x(d chunk 384): accumulate 24 K-tiles in PSUM
  ACT Copy(acc * (s2*sw2)) -> out f32 -> DMA to DRAM

Weights are quantized on the host (per-tensor scale is an init-time
constant, as sanctioned by the sharding hint), stored fp16 in DRAM in
k-tile-transposed consumption order, and streamed in 10 chunks
alternating between the GpSimd and Scalar DMA queues so both queues
pull concurrently (~2x one queue's descriptor rate).

Biases are dropped: the reference adds them in the *integer* domain
before the dequant rescale, so their relative contribution is ~1e-6 of
the integer accumulator -- far below fp32 noise in the output.
"""

import os
import sys

for _p in ("/opt/trn_rl_repo",):
    if _p not in sys.path and os.path.isdir(_p):
        sys.path.insert(0, _p)

from contextlib import ExitStack

import numpy as np

import concourse.bacc as bacc
import concourse.mybir as mybir
import concourse.tile as tile
from concourse.bass_utils import run_bass_kernel_spmd
from concourse.masks import make_identity

# Problem constants (hardcoded; kernel.py must be self-contained)
B, S, D, H = 64, 197, 768, 3072
N_CORES = 8
NTOK = B * S                      # 12608
TOK_PER_CORE = NTOK // N_CORES    # 1576
P = 128
N_TILES = (TOK_PER_CORE + P - 1) // P   # 13
TOK_PAD = N_TILES * P                   # 1664
KD = D // P                              # 6 k-tiles for fc1
KH = H // P                              # 24 k-tiles for fc2
HC = 512                                 # fc1 psum chunk (1 bank fp32)
DC = 384                                 # fc2 psum chunk (<=512)
N_HC = H // HC                           # 6
N_DC = D // DC                           # 2
NQ = 4                                   # h-quant quarters
HQ = H // NQ                             # 768 features per quarter
KHQ = KH // NQ                           # 6 k-tiles per quarter
CF16 = 1536.0                            # 1.5*2^10: fp16 RNE round trick
PE_T = 5                                 # tiles whose qx transposes on PE

F32 = mybir.dt.float32
F16 = mybir.dt.float16


def build_nc():
    nc = bacc.Bacc(
        "TRN2",
        target_bir_lowering=False,
        debug=False,
        enable_asserts=False,
        num_devices=N_CORES,
    )
    x_d = nc.dram_tensor("x", [TOK_PAD, D], F32, kind="ExternalInput").ap()
    # weights arrive pre-quantized AND pre-transposed into k-tile layout,
    # fp16, chunked to match on-device consumption order:
    # qw1t[c, p, k, j] = round(w1/sw1)[c*512+j, k*128+p]
    qw1_d = nc.dram_tensor(
        "qw1t", [N_HC, P, KD, HC], F16, kind="ExternalInput"
    ).ap()
    # qw2t[q, p, kl, d] = round(w2/sw2)[d, (q*6+kl)*128+p]
    qw2_d = nc.dram_tensor(
        "qw2t", [NQ, P, KHQ, D], F16, kind="ExternalInput"
    ).ap()
    wsc_d = nc.dram_tensor("wsc", [2], F32, kind="ExternalInput").ap()
    out_d = nc.dram_tensor("out", [TOK_PAD, D], F32, kind="ExternalOutput").ap()

    Alu = mybir.AluOpType
    Act = mybir.ActivationFunctionType

    with tile.TileContext(nc) as tc, ExitStack() as ctx:
        wpool = ctx.enter_context(tc.tile_pool(name="wpool", bufs=1))
        spool = ctx.enter_context(tc.tile_pool(name="spool", bufs=1))
        xpool = ctx.enter_context(tc.tile_pool(name="xpool", bufs=4))
        qpool = ctx.enter_context(tc.tile_pool(name="qpool", bufs=3))
        gpool = ctx.enter_context(tc.tile_pool(name="gpool", bufs=4))
        opool = ctx.enter_context(tc.tile_pool(name="opool", bufs=2))
        stpool = ctx.enter_context(tc.tile_pool(name="stpool", bufs=5))
        tpool = ctx.enter_context(tc.tile_pool(name="tpool", bufs=1, space="PSUM"))
        ps1 = ctx.enter_context(tc.tile_pool(name="ps1", bufs=5, space="PSUM"))
        ps2 = ctx.enter_context(tc.tile_pool(name="ps2", bufs=1, space="PSUM"))

        WARM = 3               # tiles whose fc1 interleaves with weight arrival

        # Weight scale broadcast first (tiny, needed by the first gsc),
        # then the identity (tiny, needed by the first PE transpose at
        # ~13us), then the weight chunks alternating across the GpSimd
        # and Scalar DMA queues in consumption order.
        wsc = spool.tile([P, 2], F32)
        import concourse.bass as bass
        wsc_bcast = bass.AP(
            tensor=wsc_d.tensor, offset=wsc_d.offset,
            ap=[[0, P]] + list(wsc_d.ap),
        )
        nc.gpsimd.dma_start(out=wsc, in_=wsc_bcast)

        # Preload the first 5 x tiles before the weight stream is issued
        # so they aren't starved by 9.4MB of weight traffic (the whole
        # in-order DVE quant chain gates on them).
        xpre = {}
        def _preload(i):
            t = xpool.tile([P, D], F32, name=f"x_{i}", tag="x_t", bufs=6)
            nc.sync.dma_start(out=t, in_=x_d[i * P:(i + 1) * P, :])
            return t
        for _i in range(min(PE_T, N_TILES)):
            xpre[_i] = _preload(_i)

        ident = spool.tile([P, P], F16)
        make_identity(nc, ident)

        wq = [nc.gpsimd, nc.scalar]
        qw1c = []
        for c in range(N_HC):
            w = wpool.tile([P, KD, HC], F16, name=f"qw1_{c}", tag=f"qw1_{c}")
            wq[c % 2].dma_start(out=w, in_=qw1_d[c])
            qw1c.append(w)
        qw2q = []
        for q in range(NQ):
            w = wpool.tile([P, KHQ, D], F16, name=f"qw2_{q}", tag=f"qw2_{q}")
            wq[q % 2].dma_start(out=w, in_=qw2_d[q])
            qw2q.append(w)

        # Later x tile loads ride the Sync HWDGE queue (which also owns
        # the steady-state xbar transposes).
        x_tiles = xpre

        # Prime the gelu ACT table set before any real work so the
        # ~2.7us table load doesn't stall the first PSUM evacuation.
        warmt = spool.tile([P, 1], F32)
        nc.scalar.activation(
            out=warmt, in_=wsc[:, 0:1], func=Act.Gelu, scale=1.0
        )

        q_ctx = {}     # i -> (qxT, gsc)
        g_ctx = {}     # i -> (g, mh6)
        state = {}     # i -> (qhT, osc)

        def quant_x(i):
            """x absmax/scales + f16 quantize + transpose for tile i."""
            x_t = x_tiles.pop(i)
            nxt = i + 2
            if nxt < N_TILES and nxt not in x_tiles and nxt >= PE_T:
                x_tiles[nxt] = _preload(nxt)

            mx = stpool.tile([P, 1], F32, name=f"mx_{i}", tag="mx")
            nc.vector.tensor_reduce(
                out=mx, in_=x_t, axis=mybir.AxisListType.X,
                op=Alu.max, apply_absolute_value=True,
            )
            s1 = stpool.tile([P, 1], F32, name=f"s1_{i}", tag="s1")
            nc.vector.tensor_scalar(
                out=s1, in0=mx, scalar1=1e-6, scalar2=1.0 / 127.0,
                op0=Alu.max, op1=Alu.mult,
            )
            rs1 = stpool.tile([P, 1], F32, name=f"rs1_{i}", tag="rs1")
            nc.vector.reciprocal(out=rs1, in_=s1)
            gsc = stpool.tile([P, 1], F32, name=f"gsc_{i}", tag="gsc", bufs=6)
            nc.vector.tensor_scalar(
                out=gsc, in0=s1, scalar1=wsc[:, 0:1], scalar2=None, op0=Alu.mult
            )
            t_x = qpool.tile([P, D], F16, name=f"tx_{i}", tag="tx")
            nc.vector.tensor_scalar(
                out=t_x, in0=x_t, scalar1=rs1, scalar2=CF16,
                op0=Alu.mult, op1=Alu.add,
            )
            qx = qpool.tile([P, D], F16, name=f"qx_{i}", tag="qx")
            nc.vector.tensor_scalar(
                out=qx, in0=t_x, scalar1=CF16, scalar2=None, op0=Alu.subtract
            )
            qxT = qpool.tile(
                [P, KD, P], F16, name=f"qxT_{i}", tag="qxT", bufs=PE_T + 1
            )
            if i < PE_T:
                # PE transpose while the DMA engines belong to weights
                st_a = tpool.tile([P, 4, P], F16, name=f"stxa_{i}", tag="st")
                for j in range(4):
                    nc.tensor.transpose(
                        st_a[:, j, :], qx[:, j * P:(j + 1) * P], ident,
                    )
                nc.vector.tensor_copy(out=qxT[:, 0:4, :], in_=st_a)
                st_b = tpool.tile([P, 4, P], F16, name=f"stxb_{i}", tag="st")
                for j in range(2):
                    nc.tensor.transpose(
                        st_b[:, j, :], qx[:, (4 + j) * P:(5 + j) * P], ident,
                    )
                nc.vector.tensor_copy(out=qxT[:, 4:6, :], in_=st_b[:, 0:2, :])
            else:
                nc.sync.dma_start(out=qxT, in_=qx, transpose=True)
            q_ctx[i] = (qxT, gsc)

        def fc1_chunk(i, hc, qxT, gsc, g, mh6):
            """One 512-wide fc1 chunk: matmul + fused scale/Gelu + amax."""
            p1 = ps1.tile([P, HC], F32, name=f"p1_{i}_{hc}", tag="p1")
            for kt in range(KD):
                nc.tensor.matmul(
                    p1,
                    lhsT=qxT[:, kt, :],
                    rhs=qw1c[hc][:, kt, :],
                    start=(kt == 0),
                    stop=(kt == KD - 1),
                )
            nc.scalar.activation(
                out=g[:, hc * HC:(hc + 1) * HC], in_=p1,
                func=Act.Gelu, scale=gsc,
            )
            nc.vector.tensor_reduce(
                out=mh6[:, hc:hc + 1], in_=g[:, hc * HC:(hc + 1) * HC],
                axis=mybir.AxisListType.X, op=Alu.max,
                apply_absolute_value=True,
            )

        def fc1(i):
            qxT, gsc = q_ctx.pop(i)
            g = gpool.tile([P, H], F32, name=f"g_{i}", tag="g")
            mh6 = stpool.tile([P, N_HC], F32, name=f"mh6_{i}", tag="mh6")
            for hc in range(N_HC):
                fc1_chunk(i, hc, qxT, gsc, g, mh6)
            g_ctx[i] = (g, mh6)

        def epilogue(i):
            """h scales + in-place f16 quantize + xbar transpose, tile i."""
            g, mh6 = g_ctx.pop(i)
            mh = stpool.tile([P, 1], F32, name=f"mh_{i}", tag="mh")
            nc.vector.tensor_reduce(
                out=mh, in_=mh6, axis=mybir.AxisListType.X, op=Alu.max
            )
            s2 = stpool.tile([P, 1], F32, name=f"s2_{i}", tag="s2")
            nc.vector.tensor_scalar(
                out=s2, in0=mh, scalar1=1e-6, scalar2=1.0 / 127.0,
                op0=Alu.max, op1=Alu.mult,
            )
            rs2 = stpool.tile([P, 1], F32, name=f"rs2_{i}", tag="rs2")
            nc.vector.reciprocal(out=rs2, in_=s2)
            osc = stpool.tile([P, 1], F32, name=f"osc_{i}", tag="osc", bufs=6)
            nc.vector.tensor_scalar(
                out=osc, in0=s2, scalar1=wsc[:, 1:2], scalar2=None, op0=Alu.mult
            )
            qh = qpool.tile([P, H], F16, name=f"qh_{i}", tag="qh", bufs=2)
            qhT = []
            for q in range(NQ):
                hs = slice(q * HQ, (q + 1) * HQ)
                nc.scalar.activation(
                    out=qh[:, hs], in_=g[:, hs], func=Act.Copy,
                    bias=CF16, scale=rs2,
                )
                nc.vector.tensor_scalar(
                    out=qh[:, hs], in0=qh[:, hs], scalar1=CF16,
                    scalar2=None, op0=Alu.subtract,
                )
                qhT_q = qpool.tile(
                    [P, KHQ, P], F16, name=f"qhT_{i}_{q}", tag=f"qhT_{q}",
                    bufs=4,
                )
                nc.sync.dma_start(out=qhT_q, in_=qh[:, hs], transpose=True)
                qhT.append(qhT_q)
            state[i] = (qhT, osc)

        def phase2(i):
            """fc2 + dequant + store for tile i."""
            qhT, osc = state.pop(i)
            o_t = opool.tile([P, D], F32, name=f"o_{i}", tag="o_t")
            p2s = [
                ps2.tile([P, DC], F32, name=f"p2_{i}_{dc}", tag=f"p2_{dc}")
                for dc in range(N_DC)
            ]
            for q in range(NQ):
                for ktl in range(KHQ):
                    kt = q * KHQ + ktl
                    for dc in range(N_DC):
                        nc.tensor.matmul(
                            p2s[dc],
                            lhsT=qhT[q][:, ktl, :],
                            rhs=qw2q[q][:, ktl, dc * DC:(dc + 1) * DC],
                            start=(kt == 0),
                            stop=(kt == KH - 1),
                        )
            for dc in range(N_DC):
                nc.scalar.activation(
                    out=o_t[:, dc * DC:(dc + 1) * DC], in_=p2s[dc],
                    func=Act.Copy, scale=osc,
                )
            nc.scalar.dma_start(out=out_d[i * P:(i + 1) * P, :], in_=o_t)

        # Prologue: pre-quantize 5 tiles (PE transposes; no weights
        # needed), interleave the first 3 tiles' fc1 hc-major so the PE
        # consumes each arriving qw1 chunk 3x back-to-back, then run
        # tile 3's fc1 to keep the PE busy while tile 0's epilogue and
        # the qw2 stream complete.
        for t in range(PE_T):
            quant_x(t)
        for t in range(WARM):
            g = gpool.tile([P, H], F32, name=f"g_{t}", tag="g")
            mh6 = stpool.tile([P, N_HC], F32, name=f"mh6_{t}", tag="mh6")
            g_ctx[t] = (g, mh6)
        for wave in range(N_HC + WARM - 1):
            for t in range(WARM):
                hc = wave - t
                if 0 <= hc < N_HC:
                    qxT, gsc = q_ctx[t]
                    g, mh6 = g_ctx[t]
                    fc1_chunk(t, hc, qxT, gsc, g, mh6)
        for t in range(WARM):
            q_ctx.pop(t)
        fc1(WARM)
        epilogue(0)

        # Steady loop: fc1 four tiles ahead, epilogue one tile ahead
        # (two ahead in the fc1-free tail so the xbar transposes lead
        # the fc2 consumer by a full tile).
        ep_done = 0   # epilogues emitted for tiles <= ep_done
        for i in range(N_TILES):
            if i + 4 < N_TILES:
                if i + 4 >= PE_T:
                    quant_x(i + 4)
                fc1(i + 4)
            targets = [i + 1]
            if i + 4 >= N_TILES:
                targets.append(i + 2)
            for j in targets:
                if ep_done < j < N_TILES:
                    epilogue(j)
                    ep_done = j
            phase2(i)

    nc.compile()
    return nc


def _host_prep(x, w1, w2):
    """Quantize + k-tile-transpose weights on the host (init constants)."""
    f32 = np.float32
    sw1 = np.maximum(np.abs(w1).max().astype(f32), f32(1e-6)) / f32(127.0)
    sw2 = np.maximum(np.abs(w2).max().astype(f32), f32(1e-6)) / f32(127.0)
    qw1 = np.round(w1.astype(f32) / sw1)   # [H, D] integers in [-127,127]
    qw2 = np.round(w2.astype(f32) / sw2)   # [D, H]
    # qw1t[c, p, k, j] = qw1[c*HC+j, k*128+p]
    qw1t = np.ascontiguousarray(
        qw1.reshape(N_HC, HC, KD, P).transpose(0, 3, 2, 1)
    ).astype(np.float16)
    # qw2t[q, p, kl, d] = qw2[d, (q*KHQ+kl)*128+p]
    qw2t = np.ascontiguousarray(
        qw2.reshape(D, NQ, KHQ, P).transpose(1, 3, 2, 0)
    ).astype(np.float16)

    x2d = np.ascontiguousarray(x.astype(f32).reshape(-1, D))
    xpad = np.zeros((N_CORES, TOK_PAD, D), dtype=np.float32)
    xpad[:, :TOK_PER_CORE, :] = x2d.reshape(N_CORES, TOK_PER_CORE, D)
    wsc = np.array([sw1, sw2], dtype=np.float32)
    return xpad, qw1t, qw2t, wsc


_NC_CACHE = []


def get_nc():
    if not _NC_CACHE:
        _NC_CACHE.append(build_nc())
    return _NC_CACHE[0]


def make_in_maps(x, w1, w2):
    xpad, qw1t, qw2t, wsc = _host_prep(x, w1, w2)
    return [
        {"x": xpad[c], "qw1t": qw1t, "qw2t": qw2t, "wsc": wsc}
        for c in range(N_CORES)
    ]


def run(nc, in_maps, **kw):
    res = run_bass_kernel_spmd(nc, in_maps, core_ids=list(range(N_CORES)), **kw)
    outs = [res.results[c]["out"][:TOK_PER_CORE] for c in range(N_CORES)]
    full = np.concatenate(outs, axis=0).reshape(B, S, D).astype(np.float32)
    return full, res


def kernel(x, w1, b1, w2, b2):
    nc = get_nc()
    in_maps = make_in_maps(np.asarray(x), np.asarray(w1), np.asarray(w2))
    full, _ = run(nc, in_maps)
    return full
